# revision 6
# baseline (speedup 1.0000x reference)
"""Trainium2 Bass kernel for BiasFreeDenoisingGNN (N=1024, H=128, E=32768, L=3).

Strategy (8 NeuronCores, one SPMD program):
  - Message passing: edges bucketed by destination window (128 dsts per core,
    host-side index bucketing). Segment-sum = one-hot(dst) matmuls on the PE
    accumulating in PSUM. Messages for a core's edges are fetched by indirect
    row gather from an HBM copy of the per-node message table. Updated node
    features are AllGathered each layer (64KB/core, overlapped).
  - Edge predictor over all 523776 upper-triangular pairs:
      h_pair @ eW1 == A[i] + B[j],  A = h @ eW1[:H], B = h @ eW1[H:]
    Pairs are processed as 512 "virtual rows" of 1024 pairs (row i fused with
    row 1022-i via a reversed copy of B so the B-side is one contiguous span).
    Per-core dynamism (which virtual rows) comes from partition_id()-driven
    dynamic slice offsets; the program is identical on all cores.
    Output is written in virtual-slot order and un-permuted on the host.
"""
import sys
import numpy as np

sys.path.insert(0, "/opt/trn_rl_repo")

import concourse.bass as bass  # noqa: E402
import concourse.bacc as bacc  # noqa: E402
import concourse.mybir as mybir  # noqa: E402
import concourse.tile as tile  # noqa: E402
from concourse.bass_utils import run_bass_kernel_spmd  # noqa: E402
from concourse.masks import make_identity  # noqa: E402
import ml_dtypes  # noqa: E402

N = 1024
H = 128
E = 32768
L = 3
C = 10
NCORES = 8
P = 128
EL = E + N          # edges incl. self loops
ECH = 36            # edge chunks of 128 per core (4608 slots, bucket max ~4500)
ESLOTS = ECH * P
VR = 64             # virtual rows per core (512 total)
VSLOTS = VR * 1024  # 65536 output slots per core
DT = mybir.dt
F32 = DT.float32
BF16 = DT.bfloat16
I32 = DT.int32
AF = mybir.ActivationFunctionType
OP = mybir.AluOpType

_CACHE = {}
LAST_RESULTS = None
TRACE = False


def _build_nc():
    nc = bacc.Bacc("TRN2", target_bir_lowering=False, debug=False,
                   enable_asserts=True, num_devices=NCORES)
    # --- kernel I/O ---
    esrc = nc.dram_tensor("esrc", [P, ECH], I32, kind="ExternalInput")
    edstl = nc.dram_tensor("edstl", [P, ECH], F32, kind="ExternalInput")
    y_f = nc.dram_tensor("y_f", [1, N], F32, kind="ExternalInput")
    t_rep = nc.dram_tensor("t_rep", [P, 1], F32, kind="ExternalInput")
    w_emb = nc.dram_tensor("w_emb", [C, H], F32, kind="ExternalInput")
    w_tw1t = nc.dram_tensor("w_tw1t", [H, 1], F32, kind="ExternalInput")
    w_tw2 = nc.dram_tensor("w_tw2", [H, H], F32, kind="ExternalInput")
    w_proj = nc.dram_tensor("w_proj", [H, H], F32, kind="ExternalInput")
    w_m1 = nc.dram_tensor("w_m1", [P, L * H], BF16, kind="ExternalInput")
    w_m2 = nc.dram_tensor("w_m2", [P, L * H], BF16, kind="ExternalInput")
    w_upd = nc.dram_tensor("w_upd", [P, L * 2 * H], BF16, kind="ExternalInput")
    w_e1t = nc.dram_tensor("w_e1t", [H, H], BF16, kind="ExternalInput")
    w_e1b = nc.dram_tensor("w_e1b", [H, H], BF16, kind="ExternalInput")
    w_e2 = nc.dram_tensor("w_e2", [H, H], BF16, kind="ExternalInput")
    w_e3 = nc.dram_tensor("w_e3", [H, 2], BF16, kind="ExternalInput")
    logits_v = nc.dram_tensor("logits_v", [VSLOTS, 2], F32, kind="ExternalOutput")
    # --- internal DRAM ---
    msgval_d = nc.dram_tensor("msgval_d", [N, H], BF16)
    ag_in = nc.dram_tensor("ag_in", [P, P], F32)
    ag_out = nc.dram_tensor("ag_out", [N, P], F32, addr_space="Shared")

    with tile.TileContext(nc) as tc:
        with tc.tile_pool(name="cst", bufs=1) as cst, \
             tc.tile_pool(name="wk", bufs=2) as wk, \
             tc.tile_pool(name="mg", bufs=4) as mgp, \
             tc.tile_pool(name="xp", bufs=3) as xp, \
             tc.tile_pool(name="rp", bufs=3) as rp, \
             tc.tile_pool(name="st", bufs=3) as stp, \
             tc.tile_pool(name="ps", bufs=2, space="PSUM") as ps, \
             tc.tile_pool(name="ps1", bufs=2, space="PSUM") as ps1:

            kreg = nc.vector.partition_id()

            # ---- load constants ----
            esrc_t = cst.tile([P, ECH], I32)
            nc.sync.dma_start(esrc_t[:], esrc[:])
            edstl_t = cst.tile([P, ECH], F32)
            nc.sync.dma_start(edstl_t[:], edstl[:])
            trep_t = cst.tile([P, 1], F32)
            nc.sync.dma_start(trep_t[:], t_rep[:])
            tw1t_t = cst.tile([H, 1], F32)
            nc.sync.dma_start(tw1t_t[:], w_tw1t[:])
            tw2_t = cst.tile([H, H], F32)
            nc.sync.dma_start(tw2_t[:], w_tw2[:])
            proj_t = cst.tile([H, H], F32)
            nc.sync.dma_start(proj_t[:], w_proj[:])
            emb_t = cst.tile([C, H], F32)
            nc.sync.dma_start(emb_t[:], w_emb[:])
            m1_t = cst.tile([P, L * H], BF16)
            nc.sync.dma_start(m1_t[:], w_m1[:])
            m2_t = cst.tile([P, L * H], BF16)
            nc.sync.dma_start(m2_t[:], w_m2[:])
            upd_t = cst.tile([P, L * 2 * H], BF16)
            nc.sync.dma_start(upd_t[:], w_upd[:])
            e1t_t = cst.tile([H, H], BF16)
            nc.sync.dma_start(e1t_t[:], w_e1t[:])
            e1b_t = cst.tile([H, H], BF16)
            nc.sync.dma_start(e1b_t[:], w_e1b[:])
            e2_t = cst.tile([H, H], BF16)
            nc.sync.dma_start(e2_t[:], w_e2[:])
            e3_t = cst.tile([H, 2], BF16)
            nc.sync.dma_start(e3_t[:], w_e3[:])
            ident = cst.tile([P, P], F32)
            make_identity(nc, ident[:])

            # ---- t embedding: t_embT[h] = (tW2.T @ relu(t * tW1.T))[h] ----
            x1t = cst.tile([H, 1], F32)
            nc.vector.tensor_scalar(out=x1t[:], in0=tw1t_t[:], scalar1=trep_t[:, :1],
                                    scalar2=0.0, op0=OP.mult, op1=OP.max)
            p_temb = ps1.tile([H, 1], F32, space="PSUM", tag="small")
            nc.tensor.matmul(out=p_temb[:], lhsT=tw2_t[:], rhs=x1t[:],
                             start=True, stop=True)
            tembT = cst.tile([H, 1], F32)
            nc.vector.tensor_copy(tembT[:], p_temb[:])

            # ---- h0 = emb[Y] via one-hot matmul (fp32, exact) ----
            yrep = cst.tile([C, N], F32)
            for c in range(C):
                nc.sync.dma_start(yrep[c:c + 1, :], y_f[0:1, :])
            iotc = cst.tile([C, 1], F32)
            nc.gpsimd.iota(iotc[:], pattern=[[0, 1]], base=0, channel_multiplier=1,
                           allow_small_or_imprecise_dtypes=True)
            oh_y = cst.tile([C, N], F32)
            nc.vector.tensor_scalar(out=oh_y[:], in0=yrep[:], scalar1=iotc[:, :1],
                                    scalar2=None, op0=OP.is_equal)
            p_h0 = ps.tile([P, N], F32, space="PSUM", tag="big")
            for half in range(2):
                sl = slice(half * 512, (half + 1) * 512)
                nc.tensor.matmul(out=p_h0[:, sl], lhsT=emb_t[:], rhs=oh_y[:, sl],
                                 start=True, stop=True)
            hpre = cst.tile([P, N], F32)
            nc.vector.tensor_scalar(out=hpre[:], in0=p_h0[:], scalar1=tembT[:, :1],
                                    scalar2=None, op0=OP.add)
            # h.T = relu(projW.T @ hpre)
            p_h = ps.tile([P, N], F32, space="PSUM", tag="big")
            for half in range(2):
                sl = slice(half * 512, (half + 1) * 512)
                nc.tensor.matmul(out=p_h[:, sl], lhsT=proj_t[:], rhs=hpre[:, sl],
                                 start=True, stop=True)
            hT = cst.tile([P, N], F32)
            nc.scalar.activation(hT[:], p_h[:], AF.Relu)
            hT_bf = cst.tile([P, N], BF16)
            nc.vector.tensor_copy(hT_bf[:], hT[:])
            # this core's dst window of h.T (f32 + bf16)
            hwin = cst.tile([P, P], F32)
            nc.vector.tensor_copy(hwin[:], hT[:, bass.ds(kreg * P, P)])
            hwin_bf = cst.tile([P, P], BF16)
            nc.vector.tensor_copy(hwin_bf[:], hwin[:])

            # ---- dst one-hot masks (once) + degrees ----
            iot128 = cst.tile([P, P], F32)
            nc.gpsimd.iota(iot128[:], pattern=[[1, P]], base=0, channel_multiplier=0,
                           allow_small_or_imprecise_dtypes=True)
            dmask = cst.tile([P, ECH * P], BF16)
            for c in range(ECH):
                nc.vector.tensor_scalar(
                    out=dmask[:, c * P:(c + 1) * P], in0=iot128[:],
                    scalar1=edstl_t[:, c:c + 1], scalar2=None, op0=OP.is_equal)
            ones_bf = cst.tile([P, 1], BF16)
            nc.vector.memset(ones_bf[:], 1.0)
            p_deg = ps1.tile([P, 1], F32, space="PSUM", tag="small")
            for c in range(ECH):
                nc.tensor.matmul(out=p_deg[:], lhsT=dmask[:, c * P:(c + 1) * P],
                                 rhs=ones_bf[:], start=(c == 0), stop=(c == ECH - 1))
            rdeg = cst.tile([P, 1], F32)
            nc.vector.reciprocal(rdeg[:], p_deg[:])

            # ---- message passing layers ----
            for l in range(L):
                # r1.T = relu(W1.T @ h.T)
                p_r1 = ps.tile([P, N], F32, space="PSUM", tag="big")
                for half in range(2):
                    sl = slice(half * 512, (half + 1) * 512)
                    nc.tensor.matmul(out=p_r1[:, sl],
                                     lhsT=m1_t[:, l * H:(l + 1) * H],
                                     rhs=hT_bf[:, sl], start=True, stop=True)
                r1_bf = wk.tile([P, N], BF16, tag="r1")
                nc.scalar.activation(r1_bf[:], p_r1[:], AF.Relu)
                # msgval rows = (r1 @ W2) : lhsT = r1.T tile
                mv_bf = wk.tile([P, N], BF16, tag="mv")
                for m in range(8):
                    p_mv = ps1.tile([P, P], F32, space="PSUM", tag="small")
                    nc.tensor.matmul(out=p_mv[:], lhsT=r1_bf[:, m * P:(m + 1) * P],
                                     rhs=m2_t[:, l * H:(l + 1) * H],
                                     start=True, stop=True)
                    nc.vector.tensor_copy(mv_bf[:, m * P:(m + 1) * P], p_mv[:])
                nc.sync.dma_start(
                    msgval_d[:].rearrange("(m p) f -> p m f", p=P),
                    mv_bf[:].rearrange("p (m f) -> p m f", m=8))
                # gather + segment-sum into this core's dst window
                p_agg = ps.tile([P, P], F32, space="PSUM", tag="pagg")
                for c in range(ECH):
                    mg_t = mgp.tile([P, P], BF16, tag="mg")
                    nc.gpsimd.indirect_dma_start(
                        out=mg_t[:], out_offset=None, in_=msgval_d[:],
                        in_offset=bass.IndirectOffsetOnAxis(
                            ap=esrc_t[:, c:c + 1], axis=0))
                    nc.tensor.matmul(out=p_agg[:], lhsT=dmask[:, c * P:(c + 1) * P],
                                     rhs=mg_t[:], start=(c == 0), stop=(c == ECH - 1))
                aggs = wk.tile([P, P], F32, tag="aggs")
                nc.vector.tensor_scalar(out=aggs[:], in0=p_agg[:],
                                        scalar1=rdeg[:, :1], scalar2=None,
                                        op0=OP.mult)
                p_at = ps1.tile([P, P], F32, space="PSUM", tag="small")
                nc.tensor.transpose(out=p_at[:], in_=aggs[:], identity=ident[:])
                aggT_bf = wk.tile([P, P], BF16, tag="aggT")
                nc.vector.tensor_copy(aggT_bf[:], p_at[:])
                # update: h_new.T window = relu(updW.T @ [h_win; agg.T]) + h_win
                p_up = ps1.tile([P, P], F32, space="PSUM", tag="small")
                base = l * 2 * H
                nc.tensor.matmul(out=p_up[:], lhsT=upd_t[:, base:base + H],
                                 rhs=hwin_bf[:], start=True, stop=False)
                nc.tensor.matmul(out=p_up[:], lhsT=upd_t[:, base + H:base + 2 * H],
                                 rhs=aggT_bf[:], start=False, stop=True)
                upr = wk.tile([P, P], F32, tag="upr")
                nc.scalar.activation(upr[:], p_up[:], AF.Relu)
                nc.vector.tensor_tensor(out=hwin[:], in0=upr[:], in1=hwin[:],
                                        op=OP.add)
                nc.vector.tensor_copy(hwin_bf[:], hwin[:])
                # AllGather windows -> full h.T
                nc.gpsimd.dma_start(ag_in[:], hwin[:])
                nc.gpsimd.collective_compute(
                    "AllGather", OP.bypass,
                    replica_groups=[list(range(NCORES))],
                    ins=[ag_in[:]], outs=[ag_out[:]])
                nc.sync.dma_start(hT[:].rearrange("p (m f) -> p m f", m=8), ag_out[:].rearrange("(m p) f -> p m f", p=P))
                nc.vector.tensor_copy(hT_bf[:], hT[:])

            # ---- predictor prep: A.T, B.T, Bext ----
            p_a = ps.tile([P, N], F32, space="PSUM", tag="big")
            for half in range(2):
                sl = slice(half * 512, (half + 1) * 512)
                nc.tensor.matmul(out=p_a[:, sl], lhsT=e1t_t[:], rhs=hT_bf[:, sl],
                                 start=True, stop=True)
            AT_f = cst.tile([P, N], F32)
            nc.vector.tensor_copy(AT_f[:], p_a[:])
            p_b = ps.tile([P, N], F32, space="PSUM", tag="big")
            for half in range(2):
                sl = slice(half * 512, (half + 1) * 512)
                nc.tensor.matmul(out=p_b[:, sl], lhsT=e1b_t[:], rhs=hT_bf[:, sl],
                                 start=True, stop=True)
            BT_bf = cst.tile([P, N], BF16)
            nc.vector.tensor_copy(BT_bf[:], p_b[:])
            bext = cst.tile([P, 2 * N], BF16)
            nc.vector.tensor_copy(bext[:, 0:N], BT_bf[:])
            nc.vector.tensor_copy(bext[:, N:2 * N], BT_bf[:, ::-1])

            # ---- predictor: 64 virtual rows of 1024 pairs ----
            for t in range(VR):
                xb = xp.tile([P, N + 512], BF16, tag="X")
                # forward row v=8t+k: X[s] = relu(A[:,v] + Bext[:, v+1+s])
                nc.vector.tensor_scalar(
                    out=xb[:, 0:N],
                    in0=bext[:, bass.ds(kreg + (8 * t + 1), N)],
                    scalar1=AT_f[:, bass.ds(kreg + 8 * t, 1)],
                    scalar2=0.0, op0=OP.add, op1=OP.max)
                # reversed row 1022-v overwrites slots [1023-v, 1023-v+512)
                nc.vector.tensor_scalar(
                    out=xb[:, bass.ds((1023 - 8 * t) - kreg, 512)],
                    in0=bext[:, N:N + 512],
                    scalar1=AT_f[:, bass.ds((1022 - 8 * t) - kreg, 1)],
                    scalar2=0.0, op0=OP.add, op1=OP.max)
                p_y = ps.tile([P, N], F32, space="PSUM", tag="big")
                nc.tensor.matmul(out=p_y[:, 0:512], lhsT=e2_t[:], rhs=xb[:, 0:512],
                                 start=True, stop=True)
                nc.tensor.matmul(out=p_y[:, 512:N], lhsT=e2_t[:], rhs=xb[:, 512:N],
                                 start=True, stop=True)
                rb = rp.tile([P, N], BF16, tag="R")
                nc.scalar.activation(rb[:], p_y[:], AF.Relu)
                p_o = ps1.tile([P, 16], F32, space="PSUM", tag="small")
                for c in range(8):
                    nc.tensor.matmul(out=p_o[:, 2 * c:2 * c + 2],
                                     lhsT=rb[:, c * P:(c + 1) * P], rhs=e3_t[:],
                                     start=True, stop=True)
                stg = stp.tile([P, 16], F32, tag="stg")
                nc.vector.tensor_copy(stg[:], p_o[:])
                nc.sync.dma_start(
                    logits_v[1024 * t:1024 * (t + 1), :].rearrange(
                        "(c p) o -> p c o", p=P),
                    stg[:].rearrange("p (c o) -> p c o", c=8))
    nc.finalize()
    return nc


def _host_prep(edge_index, Y, t_normalized, emb, tW1, tW2, projW,
               msgW1, msgW2, updW, eW1, eW2, eW3):
    bf = ml_dtypes.bfloat16
    ar = np.arange(N, dtype=np.int64)
    ei = np.concatenate([np.asarray(edge_index), np.stack([ar, ar])], axis=1)
    src = ei[0].astype(np.int64)
    dst = ei[1].astype(np.int64)
    shared = {
        "y_f": np.asarray(Y, np.float32)[None, :],
        "t_rep": np.full((P, 1), np.float32(np.asarray(t_normalized)[0])),
        "w_emb": np.asarray(emb, np.float32),
        "w_tw1t": np.asarray(tW1, np.float32).T.copy(),
        "w_tw2": np.asarray(tW2, np.float32),
        "w_proj": np.asarray(projW, np.float32),
        "w_m1": np.asarray(msgW1).astype(bf).transpose(1, 0, 2).reshape(H, L * H).copy(),
        "w_m2": np.asarray(msgW2).astype(bf).transpose(1, 0, 2).reshape(H, L * H).copy(),
        "w_upd": np.asarray(updW).astype(bf).reshape(L, 2, P, H).transpose(2, 0, 1, 3).reshape(P, L * 2 * H).copy(),
        "w_e1t": np.asarray(eW1[:H]).astype(bf),
        "w_e1b": np.asarray(eW1[H:]).astype(bf),
        "w_e2": np.asarray(eW2).astype(bf),
        "w_e3": np.asarray(eW3).astype(bf),
    }
    in_maps = []
    for k in range(NCORES):
        sel = np.nonzero((dst >> 7) == k)[0]
        cnt = len(sel)
        assert cnt <= ESLOTS, f"bucket {k} overflow: {cnt}"
        es = np.zeros(ESLOTS, np.int32)
        ed = np.full(ESLOTS, 999.0, np.float32)
        es[:cnt] = src[sel]
        ed[:cnt] = (dst[sel] - 128 * k).astype(np.float32)
        m = dict(shared)
        m["esrc"] = es.reshape(ECH, P).T.copy()
        m["edstl"] = ed.reshape(ECH, P).T.copy()
        in_maps.append(m)
    return in_maps


def _slot_to_row():
    """Map device output slot (core k, virtual row t, slot s) -> triu row id."""
    k = np.arange(NCORES)[:, None, None]
    t = np.arange(VR)[None, :, None]
    s = np.arange(1024)[None, None, :]
    v = 8 * t + k
    off = lambda i: i * 1023 - (i * (i - 1)) // 2
    fwd = s < 1023 - v
    row = np.where(fwd, off(v) + s, off(1022 - v) + (1023 - s))
    valid = fwd | ((v <= 510) & (s >= 1023 - v))
    return row, valid


def timeline_ns():
    """Cost-model timeline estimate (ns) for one core's program."""
    if "nc" not in _CACHE:
        _CACHE["nc"] = _build_nc()
        _CACHE["slotmap"] = _slot_to_row()
    from concourse.timeline_sim import TimelineSim
    return TimelineSim(_CACHE["nc"]).simulate()


def kernel(**inputs) -> np.ndarray:
    global LAST_RESULTS
    if "nc" not in _CACHE:
        _CACHE["nc"] = _build_nc()
        _CACHE["slotmap"] = _slot_to_row()
    nc = _CACHE["nc"]
    in_maps = _host_prep(**inputs)
    try:
        res = run_bass_kernel_spmd(nc, in_maps, core_ids=list(range(NCORES)),
                                   trace=TRACE)
    except ModuleNotFoundError:
        res = run_bass_kernel_spmd(nc, in_maps, core_ids=list(range(NCORES)),
                                   trace=False)
    LAST_RESULTS = res
    dev = np.stack([res.results[k]["logits_v"] for k in range(NCORES)])
    dev = dev.reshape(NCORES, VR, 1024, 2)
    row, valid = _CACHE["slotmap"]
    out = np.empty((N * (N - 1) // 2, 2), np.float32)
    out[row[valid]] = dev[valid]
    return out


if __name__ == "__main__":
    sys.path.insert(0, "/root/problem")
    import jax
    with jax.default_device(jax.devices("cpu")[0]):
        import reference
        inp = {k: np.asarray(v) for k, v in reference.setup_inputs().items()}
        exp = np.asarray(reference.reference(**reference.setup_inputs()))
    got = kernel(**inp)
    scale = np.abs(exp).max()
    err = np.abs(got - exp).max() / scale
    print("max abs:", np.abs(got - exp).max(), "scale:", scale, "rel:", err)
